# revision 8
# baseline (speedup 1.0000x reference)
"""Chamfer loss Trainium2 kernel (data-parallel over batch, 8 NeuronCores).

Problem: x, y (8, 4096, 3) fp32; loss = mean_n [ mean_w min_v ||x_nv - y_nw||
+ mean_v min_w ||x_nv - y_nw|| ] (scalar fp32).

Architecture (per core, one batch) — "quadrant soft+raw" scheme. The
4096x4096 sq-distance matrix is computed once, split into 4 quadrants of
[2048 x 2048], half x-major and half y-major:

    Q11 (v<2048, w<2048)  x-major, Exp evacuation (soft)
    Q22 (v>=2048,w>=2048) x-major, Exp evacuation (soft)
    Q12 (v<2048, w>=2048) y-major, raw fp16 evacuation (hard)
    Q21 (v>=2048,w<2048)  y-major, raw fp16 evacuation (hard)

Every x-row gets one SOFT half (ACT evacuates exp(-sq/T) with a free
fused accum_out row-sum -> softmin, zero DVE cost) and one RAW half (the
y-major quadrant's elementwise min-chain, exact). Every y-col likewise
(exp max-chain = exact pointwise hard min in exp space + y-major fold
trees). Soft halves whose row-sum underflowed (true min-sq beyond exp's
~86*T fp32 range) are replaced by +BIG via an arithmetic mask, so the
final min(soft, raw) falls back to the exact raw half; the residual
error (both halves in the far tail) measures 2.2e-3 relative on the
reference inputs at T=0.004 (gate: 2e-2).

Engine budget: ACT ~135us (60 of 64 unit evacuations + table loads),
DVE ~138us (64 col-chain TTs at fp16/bf16 2x, 28 units' fold trees
batched 4-wide via 3D APs, 4 "C" units evacuated by DVE itself through a
1x tensor_scalar with fused min-accum to offload ACT), PE ~130us (gram
matmuls + 64 epilogue transposes).

Host: packs the error-compensated 3-way bf16 split gram operands (the
augmented [24, 4096] ax/ay work in both gram orientations since
ay^T @ ax gives the transposed sq), sums the 128 output partials per
core, scales by 1/V, averages the 8 per-core losses.
"""

import sys

sys.path.insert(0, "/opt/trn_rl_repo")

from contextlib import ExitStack

import ml_dtypes
import numpy as np

import concourse.bacc as bacc
import concourse.tile as tile
from concourse import mybir
from concourse.bass_utils import run_bass_kernel_spmd

BF16 = ml_dtypes.bfloat16

P = 128
V = 4096
H = V // 2  # quadrant width (2048)
KA = 24  # augmented contraction dim (3-way hi/mid/lo split)
NMM = 512  # matmul moving free dim (one fp32 PSUM bank)
NU = 16  # units (128-row blocks) per quadrant
T_SOFT = 0.004  # softmin temperature (valid min-sq range ~86*T = 0.344)
BIG = 1.0e30
N_C = 4  # Q21 units evacuated by DVE (ACT<->DVE rebalance)

_cache = {}


def _build_nc():
    F32 = mybir.dt.float32
    F16 = mybir.dt.float16
    BF = mybir.dt.bfloat16
    mn = mybir.AluOpType.min
    mx = mybir.AluOpType.max
    X = mybir.AxisListType.X
    AF = mybir.ActivationFunctionType

    nc = bacc.Bacc("TRN2", target_bir_lowering=False)
    ax_d = nc.declare_dram_parameter("ax", [KA, V], BF, isOutput=False)
    ay_d = nc.declare_dram_parameter("ay", [KA, V], BF, isOutput=False)
    idh_d = nc.declare_dram_parameter("identh", [P, P], F16, isOutput=False)
    loss_d = nc.declare_dram_parameter("loss", [P, 1], F32, isOutput=True)

    with tile.TileContext(nc) as tc, ExitStack() as ctx:
        const = ctx.enter_context(tc.tile_pool(name="const", bufs=1))
        accs = ctx.enter_context(tc.tile_pool(name="accs", bufs=1))
        ecop = ctx.enter_context(tc.tile_pool(name="ecop", bufs=3))
        rcop = ctx.enter_context(tc.tile_pool(name="rcop", bufs=2))
        scratch = ctx.enter_context(tc.tile_pool(name="scratch", bufs=2))

        ax_sb = const.tile([KA, V], BF)
        ay_sb = const.tile([KA, V], BF)
        idh_sb = const.tile([P, P], F16)
        idb_sb = const.tile([P, P], BF)
        warmsrc = const.tile([1, 1], F32)
        warm = const.tile([1, 1], F32)
        nc.vector.memset(warmsrc[:], 1.0)
        # preload the activation table set (Sqrt/Exp/Ln) off the critical path
        nc.scalar.activation(warm[:], warmsrc[:], AF.Sqrt)
        nc.scalar.activation(warm[:], warmsrc[:], AF.Exp)
        nc.scalar.activation(warm[:], warmsrc[:], AF.Ln)
        for c in range(2):
            nc.sync.dma_start(ax_sb[:, c * H : (c + 1) * H], ax_d[:, c * H : (c + 1) * H])
            nc.sync.dma_start(ay_sb[:, c * H : (c + 1) * H], ay_d[:, c * H : (c + 1) * H])
        # identities consumed only by the epilogue transposes
        nc.sync.dma_start(idh_sb[:], idh_d[:])
        nc.vector.tensor_copy(idb_sb[:], idh_sb[:])

        # col-chain accumulators (exp space, bf16) and raw min accumulators
        cacc11 = accs.tile([P, H], BF, name="cacc11")  # max over Q11 units
        cacc22 = accs.tile([P, H], BF, name="cacc22")
        racc12 = accs.tile([P, H], F16, name="racc12")  # min over Q12 units
        racc21 = accs.tile([P, H], F16, name="racc21")
        # svals: [0:16] Q11 rowsums, [16:32] Q22 rowsums,
        #        [32:48] Q11 colmax, [48:64] Q22 colmax
        svals = accs.tile([P, 64], F32, name="svals")
        # rawf: [0:32] x-row raw halves (transposed racc12|racc21),
        #       [32:48] Q21 fold mins, [48:64] Q12 fold mins
        rawf = accs.tile([P, 64], F32, name="rawf")
        work = accs.tile([P, 4 * 64], F32, name="work")

        ESCL = -1.0 / T_SOFT

        with tc.tile_pool(name="psum", bufs=2, space="PSUM") as psum:
            # one round = one unit from each quadrant; folds batch 4 rounds
            rawq = {"q12": [], "q21": []}
            gtile = {}
            for r in range(NU):
                units = [
                    ("q11", ax_sb[:, r * P : (r + 1) * P], ay_sb[:, 0:H], cacc11, True),
                    ("q21", ay_sb[:, r * P : (r + 1) * P], ax_sb[:, H:V], racc21, False),
                    ("q22", ax_sb[:, H + r * P : H + (r + 1) * P], ay_sb[:, H:V], cacc22, True),
                    ("q12", ay_sb[:, H + r * P : H + (r + 1) * P], ax_sb[:, 0:H], racc12, False),
                ]
                for qi, (qn, lhsT, rhs, acc, is_exp) in enumerate(units):
                    pst = psum.tile([P, H], F32, name="ps", tag="ps")
                    for j in range(H // NMM):
                        nc.tensor.matmul(
                            pst[:, j * NMM : (j + 1) * NMM],
                            lhsT,
                            rhs[:, j * NMM : (j + 1) * NMM],
                            start=True,
                            stop=True,
                        )
                    if is_exp:
                        # soft half: exp evac + free fused row-sum
                        slot = (0 if qn == "q11" else 16) + r
                        ct = ecop.tile([P, H], BF, name="cte", tag="cte")
                        nc.scalar.activation(
                            ct[:], pst[:], AF.Exp, scale=ESCL,
                            accum_out=svals[:, slot : slot + 1],
                        )
                        if r == 0:
                            nc.vector.tensor_copy(acc[:], ct[:])
                        else:
                            nc.vector.tensor_tensor(acc[:], ct[:], acc[:], mx)
                    else:
                        slot = (32 if qn == "q21" else 48) + r
                        is_c = qn == "q21" and r >= NU - N_C
                        if r % 4 == 0:
                            rawq[qn] = []
                            gtile[qn] = rcop.tile(
                                [P, 4 * H], F16, name="ctr", tag=f"ctr{qn}"
                            )
                        ct = gtile[qn]
                        ctu = ct[:, (r % 4) * H : (r % 4 + 1) * H]
                        if is_c:
                            # DVE evacuation with fused min-accum row fold
                            nc.vector.tensor_scalar(
                                out=ctu, in0=pst[:], scalar1=BIG, scalar2=None,
                                op0=mn, op1=mn,
                                accum_out=rawf[:, slot : slot + 1],
                            )
                        else:
                            # raw fp16 evac; fold trees batched per 4 units
                            nc.scalar.copy(ctu, pst[:])
                            rawq[qn].append((r, ct))
                        if r == 0:
                            nc.vector.tensor_copy(acc[:], ctu)
                        else:
                            nc.vector.tensor_tensor(acc[:], ctu, acc[:], mn)

                # batched fold trees for completed groups of 4 raw units
                if r % 4 == 3:
                    for qn in ("q21", "q12"):
                        grp = rawq[qn]
                        if not grp:
                            continue
                        r0, ct = grp[0][0], grp[0][1]
                        base = 32 if qn == "q21" else 48
                        c3 = ct[:].rearrange("p (m w) -> p m w", m=4)
                        scr = scratch.tile([P, 4 * (H // 2)], F16, name="scr", tag="scr")
                        s3 = scr[:].rearrange("p (m w) -> p m w", m=4)
                        nc.vector.tensor_tensor(
                            s3[:, :, :], c3[:, :, : H // 2], c3[:, :, H // 2 :], mn
                        )
                        nc.vector.tensor_tensor(
                            s3[:, :, : H // 4], s3[:, :, : H // 4],
                            s3[:, :, H // 4 : H // 2], mn,
                        )
                        nc.vector.tensor_tensor(
                            s3[:, :, : H // 8], s3[:, :, : H // 8],
                            s3[:, :, H // 8 : H // 4], mn,
                        )
                        nc.vector.tensor_reduce(
                            rawf[:, base + r0 : base + r0 + 4],
                            s3[:, :, : H // 8],
                            axis=X,
                            op=mn,
                        )

        # Epilogue: transpose the 4 accumulators, partition-reduce, combine
        with tc.tile_pool(name="psum_ep", bufs=1, space="PSUM") as psum_ep:
            tpr = psum_ep.tile([P, 2 * H], F16, name="tpr")
            tpe = psum_ep.tile([P, 2 * H], BF, name="tpe")
            for b in range(NU):
                nc.tensor.transpose(
                    tpr[:, b * P : (b + 1) * P], racc12[:, b * P : (b + 1) * P], idh_sb[:]
                )
                nc.tensor.transpose(
                    tpr[:, H + b * P : H + (b + 1) * P], racc21[:, b * P : (b + 1) * P], idh_sb[:]
                )
                nc.tensor.transpose(
                    tpe[:, b * P : (b + 1) * P], cacc11[:, b * P : (b + 1) * P], idb_sb[:]
                )
                nc.tensor.transpose(
                    tpe[:, H + b * P : H + (b + 1) * P], cacc22[:, b * P : (b + 1) * P], idb_sb[:]
                )
            # x-row raw halves: [0:16] v<2048 (racc12), [16:32] v>=2048 (racc21)
            nc.vector.tensor_reduce(
                rawf[:, 0:32], tpr[:].rearrange("p (a b) -> p a b", a=2 * NU),
                axis=X, op=mn,
            )
            # y-col soft halves: [32:48] w<2048 (cacc11), [48:64] w>=2048
            nc.vector.tensor_reduce(
                svals[:, 32:64], tpe[:].rearrange("p (a b) -> p a b", a=2 * NU),
                axis=X, op=mx,
            )

            # soft mapping: sq_soft = -T * ln(s), replaced by SATBIG (> any
            # true sq) where the exp sum/max underflowed; then min with raw.
            # Ln's spline clamps below ~1e-20, so feed it s*2^64 and subtract
            # T*64*ln2; SATBIG is small (128) to avoid fp32 absorption of the
            # soft value in the blend arithmetic.
            SATBIG = 128.0
            lnv = work[:, 0:64]
            mask = work[:, 64:128]
            blend = work[:, 128:192]
            sfloor = work[:, 192:256]
            nc.vector.tensor_scalar(
                out=sfloor, in0=svals[:], scalar1=float(2.0**64),
                scalar2=1e-19, op0=mybir.AluOpType.mult, op1=mx,
            )
            nc.scalar.activation(lnv, sfloor, AF.Ln)
            # mask = min(s * 1e36, 1): ~0 for underflowed slots, 1 otherwise
            nc.vector.tensor_scalar(
                out=mask, in0=svals[:], scalar1=1e36, scalar2=1.0,
                op0=mybir.AluOpType.mult, op1=mn,
            )
            # blend = (-T*lnv + T*64*ln2) * mask + (1 - mask) * SATBIG
            nc.vector.tensor_scalar(
                out=blend, in0=lnv, scalar1=-T_SOFT,
                scalar2=float(T_SOFT * 64.0 * np.log(2.0)),
                op0=mybir.AluOpType.mult, op1=mybir.AluOpType.add,
            )
            nc.vector.tensor_tensor(blend, blend, mask, mybir.AluOpType.mult)
            nc.vector.tensor_scalar(
                out=mask, in0=mask, scalar1=-SATBIG, scalar2=SATBIG,
                op0=mybir.AluOpType.mult, op1=mybir.AluOpType.add,
            )
            nc.vector.tensor_tensor(blend, blend, mask, mybir.AluOpType.add)
            fin = work[:, 0:64]
            nc.vector.tensor_tensor(fin, blend, rawf[:], mn)
            nc.vector.tensor_scalar(
                out=fin, in0=fin, scalar1=0.0, scalar2=None, op0=mx
            )
            stot = accs.tile([P, 1], F32, name="stot")
            nc.scalar.activation(fin, fin, AF.Sqrt, accum_out=stot[:])
            nc.sync.dma_start(loss_d[:], stot[:])

    nc.finalize()
    return nc


def _split3(v):
    """3-way bf16 split: v ~= h + m + l with residual ~2^-27 |v|."""
    f32 = np.float32
    h = v.astype(BF16)
    m = (v - h.astype(f32)).astype(BF16)
    l = (v - h.astype(f32) - m.astype(f32)).astype(BF16)
    return h, m, l


def _augment(x, y):
    """x, y: (V, 3) fp32 -> AX, AY [24, V] bf16 3-way-split gram operands.

    sq = x2 + y2 + x.(-2y); products kept: hh, hm, mh, hl, lh, mm
    (magnitude >= ~2^-16); x2/y2 carried as 3 bf16 rows each. Symmetric:
    ay^T @ ax yields the transposed sq, so the same operands serve both
    gram orientations.
    """
    f32 = np.float32
    yy = (-2.0 * y).astype(f32)
    xh, xm, xl = _split3(x)
    yh, ym, yl = _split3(yy)
    x2 = np.einsum("vc,vc->v", x.astype(np.float64), x.astype(np.float64)).astype(f32)
    y2 = np.einsum("vc,vc->v", y.astype(np.float64), y.astype(np.float64)).astype(f32)
    x2h, x2m, x2l = _split3(x2)
    y2h, y2m, y2l = _split3(y2)
    one = np.ones(V, dtype=BF16)

    def cols(a):
        return [a[:, 0], a[:, 1], a[:, 2]]

    ax = np.stack(
        cols(xh) + cols(xh) + cols(xm) + cols(xh) + cols(xl) + cols(xm)
        + [x2h, x2m, x2l, one, one, one]
    )
    ay = np.stack(
        cols(yh) + cols(ym) + cols(yh) + cols(yl) + cols(yh) + cols(ym)
        + [one, one, one, y2h, y2m, y2l]
    )
    return ax, ay


def kernel(x, y):
    x = np.asarray(x, dtype=np.float32)
    y = np.asarray(y, dtype=np.float32)
    n = x.shape[0]
    assert x.shape == (n, V, 3) and y.shape == (n, V, 3) and n == 8

    if "nc" not in _cache:
        _cache["nc"] = _build_nc()
    nc = _cache["nc"]

    identh = np.eye(P, dtype=np.float16)
    in_maps = []
    for i in range(n):
        ax, ay = _augment(x[i], y[i])
        in_maps.append({"ax": ax, "ay": ay, "identh": identh})

    res = run_bass_kernel_spmd(
        nc, in_maps, list(range(n)), trace=_cache.get("trace", False)
    )
    _cache["last"] = res
    scale = 1.0 / V
    vals = [
        np.asarray(res.results[i]["loss"], dtype=np.float64).sum() * scale
        for i in range(n)
    ]
    return np.asarray(np.mean(vals), dtype=np.float32)
